# revision 12
# baseline (speedup 1.0000x reference)
"""Trainium2 Bass kernel for nn_JinaPairTraining (dense CE + late-interaction
maxsim CE + KL between the two softmax distributions).

Sharding: data-parallel over the query batch dim Bq. Rows are assigned to the
8 cores to balance valid-q-token counts; every core receives the full
(mask-packed) pos side and computes its rows of the raw maxsim matrix
S_raw[row, doc] = sum_{valid q} max_{valid p} sim.  The host does everything
else: the dense [32,32] logits (tiny), the row softmax / CE / KL in float64,
and the final mean.  Only the O(B^2 T^2 D) sim work runs on device.

Mask packing (exact, no approximation):
  * q side: only valid q tokens are shipped, packed into chunks of 128
    (crossing row boundaries).  The masked one-hot stationary (qoh) of the
    final sum-over-q matmul routes each token slot to its row; pad slots get
    weight 0.
  * p side: only valid pos tokens are shipped.  Tokens are pair-folded
    (max(s0, s1) = s1 + relu(s0 - s1), computed as PE matmuls + one ACT relu
    + an identity-matmul accumulate).  Docs are sorted by pair count and
    grouped into 4 regions of 8 docs; each region pads its docs to the
    region max with duplicate pairs (duplicates never change a max).
  * the kernel is compiled per (chunk-count, region-widths) signature and
    cached; all-ones masks degenerate to the dense full-size layout.
"""

import os
import sys

import numpy as np

for _p in ("/opt/trn_rl_repo",):
    if _p not in sys.path and os.path.isdir(_p):
        sys.path.insert(0, _p)

import concourse.bacc as bacc
import concourse.tile as tile
from concourse import mybir
from concourse.bass_utils import run_bass_kernel_spmd

B, T, D = 32, 256, 128
TAU = 0.02
EPS = 1e-8
NCORES = 8
BPC = B // NCORES  # 4 query rows per core
NREG = 4           # pos regions (8 docs each, sorted by valid-pair count)
DPR = B // NREG    # docs per region

F32 = mybir.dt.float32
BF16 = mybir.dt.bfloat16
AX = mybir.AxisListType
ACT = mybir.ActivationFunctionType


def _build_kernel(nj, widths):
    """nj: q chunks per core; widths: per-region pairs-per-doc (s_r)."""
    nc = bacc.Bacc(None, target_bir_lowering=False, debug=False)

    totw = sum(DPR * s for s in widths)
    pT_d = nc.dram_tensor("pT", [D, 2 * totw], BF16, kind="ExternalInput")
    qT_d = nc.dram_tensor("qT", [D, nj * 128], BF16, kind="ExternalInput")
    ident_d = nc.dram_tensor("identity", [128, 128], BF16, kind="ExternalInput")
    qoh_d = nc.dram_tensor("qoh", [D, nj, BPC], F32, kind="ExternalInput")
    out_d = nc.dram_tensor("out", [BPC, B], F32, kind="ExternalOutput")

    roff = np.cumsum([0] + [2 * DPR * s for s in widths]).tolist()

    with tile.TileContext(nc) as tc:
        with tc.tile_pool(name="sb", bufs=1) as sb:
            # qT + smalls ride the ACT queue; p regions stream on the SP
            # queue in parallel.
            qT = sb.tile([D, nj * 128], BF16)
            nc.scalar.dma_start(out=qT, in_=qT_d[:, :])
            pT = sb.tile([D, 2 * totw], BF16)
            for r in range(NREG):
                eng = nc.sync if r % 2 == 0 else nc.scalar
                eng.dma_start(
                    out=pT[:, roff[r] : roff[r + 1]],
                    in_=pT_d[:, roff[r] : roff[r + 1]],
                )
            ident = sb.tile([128, 128], BF16)
            nc.sync.dma_start(out=ident, in_=ident_d[:, :])
            qoh = sb.tile([D, nj, BPC], F32)
            nc.sync.dma_start(out=qoh, in_=qoh_d[:, :, :])

            # mx[q, j, c]: per q chunk j, per pos doc c (sorted order), the
            # masked max over that doc's tokens.
            mx = sb.tile([128, nj, B], F32)

            with (
                tc.tile_pool(name="pb", bufs=2, space="PSUM") as pb,
                tc.tile_pool(name="rp", bufs=2) as rp,
            ):
                for r in range(NREG):
                    w = DPR * widths[r]
                    pd = pT[:, roff[r] : roff[r] + w]
                    p1 = pT[:, roff[r] + w : roff[r + 1]]
                    nchunk = (w + 511) // 512
                    for j in range(nj):
                        qj = qT[:, j * 128 : (j + 1) * 128]
                        ps_d = pb.tile([128, w], F32, name="ps_d")
                        for k in range(nchunk):
                            sl = slice(k * 512, min((k + 1) * 512, w))
                            nc.tensor.matmul(
                                ps_d[:, sl], qj, pd[:, sl], start=True, stop=True
                            )
                        ps_m = pb.tile([128, w], F32, name="ps_m")
                        for k in range(nchunk):
                            sl = slice(k * 512, min((k + 1) * 512, w))
                            nc.tensor.matmul(
                                ps_m[:, sl], qj, p1[:, sl], start=True, stop=False
                            )
                        relu_sb = rp.tile([128, w], BF16, name="relu_sb")
                        nc.scalar.activation(relu_sb, ps_d, ACT.Relu)
                        for k in range(nchunk):
                            sl = slice(k * 512, min((k + 1) * 512, w))
                            nc.tensor.matmul(
                                ps_m[:, sl],
                                ident,
                                relu_sb[:, sl],
                                start=False,
                                stop=True,
                            )
                        nc.vector.reduce_max(
                            out=mx[:, j, r * DPR : (r + 1) * DPR],
                            in_=ps_m.rearrange("p (g s) -> p g s", s=widths[r]),
                            axis=AX.X,
                        )

            # S_raw rows: s_ps[row, doc] = sum_j qoh_j^T @ mx_j
            with tc.tile_pool(name="pss", bufs=1, space="PSUM") as pss:
                s_ps = pss.tile([BPC, B], F32)
                for j in range(nj):
                    nc.tensor.matmul(
                        s_ps,
                        qoh[:, j, :],
                        mx[:, j, :],
                        start=(j == 0),
                        stop=(j == nj - 1),
                    )
                out_sb = sb.tile([BPC, B], F32)
                nc.vector.tensor_copy(out_sb, s_ps)
                nc.sync.dma_start(out=out_d[:, :], in_=out_sb)

    nc.compile()
    return nc


_NC_CACHE = {}
_LAST_NC = None


def _get_nc(nj=None, widths=None):
    global _LAST_NC
    if nj is None:
        return _LAST_NC
    key = (nj, tuple(widths))
    if key not in _NC_CACHE:
        _NC_CACHE[key] = _build_kernel(nj, widths)
    _LAST_NC = _NC_CACHE[key]
    return _LAST_NC


def _pad4(x):
    return (x + 3) & ~3


def _plan(q_mask, p_mask):
    """Row->core assignment, q chunk count, pos doc order + region widths."""
    qlen = q_mask.sum(axis=1).astype(int)
    # Balance valid-q counts across cores (4 rows each): greedy LPT, then
    # pairwise-swap refinement to minimize the max core sum (which sets the
    # compiled chunk count for every core).
    order = np.argsort(-qlen, kind="stable")
    sums = [0] * NCORES
    counts = [0] * NCORES
    rows_per_core = [[] for _ in range(NCORES)]
    for b in order:
        cands = [c for c in range(NCORES) if counts[c] < BPC]
        c = min(cands, key=lambda c: sums[c])
        rows_per_core[c].append(int(b))
        sums[c] += int(qlen[b])
        counts[c] += 1
    improved = True
    while improved:
        improved = False
        hi = int(np.argmax(sums))
        for lo in sorted(range(NCORES), key=lambda c: sums[c]):
            if lo == hi:
                continue
            for i, bh in enumerate(rows_per_core[hi]):
                for k, bl in enumerate(rows_per_core[lo]):
                    delta = int(qlen[bh]) - int(qlen[bl])
                    if delta <= 0:
                        continue
                    new_hi = sums[hi] - delta
                    new_lo = sums[lo] + delta
                    if max(new_hi, new_lo) < sums[hi]:
                        rows_per_core[hi][i], rows_per_core[lo][k] = bl, bh
                        sums[hi], sums[lo] = new_hi, new_lo
                        improved = True
                        break
                if improved:
                    break
            if improved:
                break
    nj = max(1, (max(sums) + 127) // 128)

    # Pos docs sorted by valid-pair count, 4 regions of 8.
    plen = p_mask.sum(axis=1).astype(int)
    pairs = (plen + 1) // 2
    doc_order = np.argsort(pairs, kind="stable")
    widths = []
    for r in range(NREG):
        grp = doc_order[r * DPR : (r + 1) * DPR]
        widths.append(int(_pad4(max(1, pairs[grp].max()))))
    return rows_per_core, nj, doc_order, widths


def _prep_pos(pm, pmask, doc_order, widths):
    """Packed [D, 2*totw] bf16 pos tensor: per region [pd block | p1 block]."""
    import ml_dtypes

    blocks = []
    for r in range(NREG):
        s = widths[r]
        pd_blk = np.zeros((DPR * s, D), np.float32)
        p1_blk = np.zeros((DPR * s, D), np.float32)
        for i, c in enumerate(doc_order[r * DPR : (r + 1) * DPR]):
            tok = pm[c][pmask[c]]  # [L, D] valid tokens
            L = len(tok)
            if L == 1:
                pa, pb_ = tok.copy(), tok.copy()
            else:
                h = L // 2
                pairs_a = [tok[:h]]
                pairs_b = [tok[h : 2 * h]]
                if L % 2 == 1:
                    pairs_a.append(tok[L - 1 : L])
                    pairs_b.append(tok[0:1])
                pa = np.concatenate(pairs_a, axis=0)
                pb_ = np.concatenate(pairs_b, axis=0)
            npair = len(pa)
            # pad with duplicates of pair 0
            pad = s - npair
            if pad > 0:
                pa = np.concatenate([pa, np.repeat(pa[0:1], pad, axis=0)], axis=0)
                pb_ = np.concatenate([pb_, np.repeat(pb_[0:1], pad, axis=0)], axis=0)
            pd_blk[i * s : (i + 1) * s] = pa - pb_
            p1_blk[i * s : (i + 1) * s] = pb_
        blocks.append(pd_blk.T)
        blocks.append(p1_blk.T)
    pT = np.ascontiguousarray(np.concatenate(blocks, axis=1)).astype(
        ml_dtypes.bfloat16
    )
    return pT


def _prep_in_maps(query_multi, pos_multi, q_mask, p_mask, plan):
    import ml_dtypes

    rows_per_core, nj, doc_order, widths = plan
    qm = np.ascontiguousarray(np.asarray(query_multi, np.float32))
    pm = np.ascontiguousarray(np.asarray(pos_multi, np.float32))
    qmask = np.asarray(q_mask).astype(bool)
    pmask = np.asarray(p_mask).astype(bool)

    pT = _prep_pos(pm, pmask, doc_order, widths)
    ident = np.eye(128, dtype=ml_dtypes.bfloat16)

    in_maps = []
    for c in range(NCORES):
        qtok = np.zeros((nj * 128, D), np.float32)
        qoh = np.zeros((nj * 128, BPC), np.float32)
        pos = 0
        for i, b in enumerate(rows_per_core[c]):
            tok = qm[b][qmask[b]]
            n = len(tok)
            qtok[pos : pos + n] = tok
            qoh[pos : pos + n, i] = 1.0
            pos += n
        qT = np.ascontiguousarray(qtok.T).astype(ml_dtypes.bfloat16)
        qoh3 = np.ascontiguousarray(qoh.reshape(nj, 128, BPC).transpose(1, 0, 2))
        in_maps.append({"pT": pT, "qT": qT, "identity": ident, "qoh": qoh3})
    return in_maps


def _host_losses(dense_sim, S_late):
    """Float64 replica of the reference softmax/CE/KL tail."""

    def softmax_and_logp(z):
        m = z.max(axis=1, keepdims=True)
        e = np.exp(z - m)
        den = e.sum(axis=1, keepdims=True)
        return e / den, (z - m) - np.log(den)

    zd = dense_sim / TAU
    zl = S_late / TAU
    dp, logp_d = softmax_and_logp(zd)
    lp, logp_l = softmax_and_logp(zl)
    idx = np.arange(B)
    single = -logp_d[idx, idx].mean()
    multi = -logp_l[idx, idx].mean()
    kl = (dp * np.log((dp + EPS) / (lp + EPS))).sum(axis=1).mean()
    return single, multi, kl


def run(inputs: dict, trace: bool = False):
    """Run the spmd kernel; returns (loss tuple, BassKernelResults)."""
    qmask = np.asarray(inputs["q_mask"]).astype(bool)
    pmask = np.asarray(inputs["p_mask"]).astype(bool)
    plan = _plan(qmask, pmask)
    rows_per_core, nj, doc_order, widths = plan

    nc = _get_nc(nj, widths)
    in_maps = _prep_in_maps(
        inputs["query_multi"], inputs["pos_multi"], qmask, pmask, plan
    )
    res = run_bass_kernel_spmd(nc, in_maps, core_ids=list(range(NCORES)), trace=trace)

    # Assemble S_raw in original (row, doc) order.
    S_raw = np.zeros((B, B), np.float64)
    inv_doc = np.argsort(doc_order)
    for c in range(NCORES):
        block = np.asarray(res.results[c]["out"], np.float64)  # [BPC, B]
        for i, b in enumerate(rows_per_core[c]):
            S_raw[b] = block[i][inv_doc]

    t_i = np.maximum(qmask.sum(axis=1), 1).astype(np.float64)
    S_late = S_raw / t_i[:, None]

    qs = np.asarray(inputs["query_single"], np.float64)
    ps = np.asarray(inputs["pos_single"], np.float64)
    dense_sim = qs @ ps.T

    single, multi, kl = _host_losses(dense_sim, S_late)
    total = single + multi + kl
    out = (np.float32(total), np.float32(single), np.float32(multi), np.float32(kl))
    return out, res


def kernel(query_single, pos_single, query_multi, pos_multi, q_mask, p_mask):
    out, _ = run(
        {
            "query_single": query_single,
            "pos_single": pos_single,
            "query_multi": query_multi,
            "pos_multi": pos_multi,
            "q_mask": q_mask,
            "p_mask": p_mask,
        }
    )
    return out


# revision 13
# speedup vs baseline: 1.0221x; 1.0221x over previous
"""Trainium2 Bass kernel for nn_JinaPairTraining (dense CE + late-interaction
maxsim CE + KL between the two softmax distributions).

Sharding: data-parallel over the query batch dim Bq. Rows are assigned to the
8 cores to balance valid-q-token counts; every core receives the full
(mask-packed) pos side and computes its rows of the raw maxsim matrix
S_raw[row, doc] = sum_{valid q} max_{valid p} sim.  The host does everything
else: the dense [32,32] logits (tiny), the row softmax / CE / KL in float64,
and the final mean.  Only the O(B^2 T^2 D) sim work runs on device.

Mask packing (exact, no approximation):
  * q side: only valid q tokens are shipped, packed into chunks of 128
    (crossing row boundaries).  The masked one-hot stationary (qoh) of the
    final sum-over-q matmul routes each token slot to its row; pad slots get
    weight 0.
  * p side: only valid pos tokens are shipped.  Tokens are pair-folded
    (max(s0, s1) = s1 + relu(s0 - s1), computed as PE matmuls + one ACT relu
    + an identity-matmul accumulate).  Docs are sorted by pair count and
    grouped into 4 regions of 8 docs; each region pads its docs to the
    region max with duplicate pairs (duplicates never change a max).
  * the kernel is compiled per (chunk-count, region-widths) signature and
    cached; all-ones masks degenerate to the dense full-size layout.
"""

import os
import sys

import numpy as np

for _p in ("/opt/trn_rl_repo",):
    if _p not in sys.path and os.path.isdir(_p):
        sys.path.insert(0, _p)

import concourse.bacc as bacc
import concourse.tile as tile
from concourse import mybir
from concourse.bass_utils import run_bass_kernel_spmd

B, T, D = 32, 256, 128
TAU = 0.02
EPS = 1e-8
NCORES = 8
BPC = B // NCORES  # 4 query rows per core
NREG = 4           # pos regions (8 docs each, sorted by valid-pair count)
DPR = B // NREG    # docs per region

F32 = mybir.dt.float32
BF16 = mybir.dt.bfloat16
AX = mybir.AxisListType
ACT = mybir.ActivationFunctionType


def _build_kernel(nj, widths):
    """nj: q chunks per core; widths: per-region pairs-per-doc (s_r)."""
    nc = bacc.Bacc(None, target_bir_lowering=False, debug=False)

    totw = sum(DPR * s for s in widths)
    pT_d = nc.dram_tensor("pT", [D, 2 * totw], BF16, kind="ExternalInput")
    qT_d = nc.dram_tensor("qT", [D, nj * 128], BF16, kind="ExternalInput")
    ident_d = nc.dram_tensor("identity", [128, 128], BF16, kind="ExternalInput")
    qoh_d = nc.dram_tensor("qoh", [D, nj, BPC], F32, kind="ExternalInput")
    out_d = nc.dram_tensor("out", [BPC, B], F32, kind="ExternalOutput")

    roff = np.cumsum([0] + [2 * DPR * s for s in widths]).tolist()

    with tile.TileContext(nc) as tc:
        with tc.tile_pool(name="sb", bufs=1) as sb:
            # qT + smalls ride the ACT queue; p regions stream on the SP
            # queue in parallel.
            qT = sb.tile([D, nj * 128], BF16)
            nc.scalar.dma_start(out=qT, in_=qT_d[:, :])
            pT = sb.tile([D, 2 * totw], BF16)
            for r in range(NREG):
                eng = nc.sync if r % 2 == 0 else nc.scalar
                eng.dma_start(
                    out=pT[:, roff[r] : roff[r + 1]],
                    in_=pT_d[:, roff[r] : roff[r + 1]],
                )
            ident = sb.tile([128, 128], BF16)
            nc.sync.dma_start(out=ident, in_=ident_d[:, :])
            qoh = sb.tile([D, nj, BPC], F32)
            nc.sync.dma_start(out=qoh, in_=qoh_d[:, :, :])

            # mx[q, j, c]: per q chunk j, per pos doc c (sorted order), the
            # masked max over that doc's tokens.
            mx = sb.tile([128, nj, B], F32)

            with (
                tc.tile_pool(name="pb", bufs=2, space="PSUM") as pb,
                tc.tile_pool(name="rp", bufs=2) as rp,
            ):
                for r in range(NREG):
                    w = DPR * widths[r]
                    pd = pT[:, roff[r] : roff[r] + w]
                    p1 = pT[:, roff[r] + w : roff[r + 1]]
                    nchunk = (w + 511) // 512
                    for j in range(nj):
                        qj = qT[:, j * 128 : (j + 1) * 128]
                        ps_d = pb.tile([128, w], F32, name="ps_d")
                        for k in range(nchunk):
                            sl = slice(k * 512, min((k + 1) * 512, w))
                            nc.tensor.matmul(
                                ps_d[:, sl], qj, pd[:, sl], start=True, stop=True
                            )
                        # ps_m as two 4-doc tiles (each <= 1 PSUM bank) so
                        # each is freed right after its own reduce.
                        hw_ = w // 2
                        ps_ms = [
                            pb.tile([128, hw_], F32, name=f"ps_m{h}")
                            for h in range(2)
                        ]
                        for h in range(2):
                            nc.tensor.matmul(
                                ps_ms[h],
                                qj,
                                p1[:, h * hw_ : (h + 1) * hw_],
                                start=True,
                                stop=False,
                            )
                        relu_sb = rp.tile([128, w], BF16, name="relu_sb")
                        nc.scalar.activation(relu_sb, ps_d, ACT.Relu)
                        for h in range(2):
                            nc.tensor.matmul(
                                ps_ms[h],
                                ident,
                                relu_sb[:, h * hw_ : (h + 1) * hw_],
                                start=False,
                                stop=True,
                            )
                        for h in range(2):
                            nc.vector.reduce_max(
                                out=mx[
                                    :,
                                    j,
                                    r * DPR + h * (DPR // 2) : r * DPR
                                    + (h + 1) * (DPR // 2),
                                ],
                                in_=ps_ms[h].rearrange(
                                    "p (g s) -> p g s", s=widths[r]
                                ),
                                axis=AX.X,
                            )

            # S_raw rows: s_ps[row, doc] = sum_j qoh_j^T @ mx_j
            with tc.tile_pool(name="pss", bufs=1, space="PSUM") as pss:
                s_ps = pss.tile([BPC, B], F32)
                for j in range(nj):
                    nc.tensor.matmul(
                        s_ps,
                        qoh[:, j, :],
                        mx[:, j, :],
                        start=(j == 0),
                        stop=(j == nj - 1),
                    )
                out_sb = sb.tile([BPC, B], F32)
                nc.vector.tensor_copy(out_sb, s_ps)
                nc.sync.dma_start(out=out_d[:, :], in_=out_sb)

    nc.compile()
    return nc


_NC_CACHE = {}
_LAST_NC = None


def _get_nc(nj=None, widths=None):
    global _LAST_NC
    if nj is None:
        return _LAST_NC
    key = (nj, tuple(widths))
    if key not in _NC_CACHE:
        _NC_CACHE[key] = _build_kernel(nj, widths)
    _LAST_NC = _NC_CACHE[key]
    return _LAST_NC


def _pad4(x):
    return (x + 3) & ~3


def _plan(q_mask, p_mask):
    """Row->core assignment, q chunk count, pos doc order + region widths."""
    qlen = q_mask.sum(axis=1).astype(int)
    # Balance valid-q counts across cores (4 rows each): greedy LPT, then
    # pairwise-swap refinement to minimize the max core sum (which sets the
    # compiled chunk count for every core).
    order = np.argsort(-qlen, kind="stable")
    sums = [0] * NCORES
    counts = [0] * NCORES
    rows_per_core = [[] for _ in range(NCORES)]
    for b in order:
        cands = [c for c in range(NCORES) if counts[c] < BPC]
        c = min(cands, key=lambda c: sums[c])
        rows_per_core[c].append(int(b))
        sums[c] += int(qlen[b])
        counts[c] += 1
    improved = True
    while improved:
        improved = False
        hi = int(np.argmax(sums))
        for lo in sorted(range(NCORES), key=lambda c: sums[c]):
            if lo == hi:
                continue
            for i, bh in enumerate(rows_per_core[hi]):
                for k, bl in enumerate(rows_per_core[lo]):
                    delta = int(qlen[bh]) - int(qlen[bl])
                    if delta <= 0:
                        continue
                    new_hi = sums[hi] - delta
                    new_lo = sums[lo] + delta
                    if max(new_hi, new_lo) < sums[hi]:
                        rows_per_core[hi][i], rows_per_core[lo][k] = bl, bh
                        sums[hi], sums[lo] = new_hi, new_lo
                        improved = True
                        break
                if improved:
                    break
            if improved:
                break
    nj = max(1, (max(sums) + 127) // 128)

    # Pos docs sorted by valid-pair count, 4 regions of 8.
    plen = p_mask.sum(axis=1).astype(int)
    pairs = (plen + 1) // 2
    doc_order = np.argsort(pairs, kind="stable")
    widths = []
    for r in range(NREG):
        grp = doc_order[r * DPR : (r + 1) * DPR]
        widths.append(int(_pad4(max(1, pairs[grp].max()))))
    return rows_per_core, nj, doc_order, widths


def _prep_pos(pm, pmask, doc_order, widths):
    """Packed [D, 2*totw] bf16 pos tensor: per region [pd block | p1 block]."""
    import ml_dtypes

    blocks = []
    for r in range(NREG):
        s = widths[r]
        pd_blk = np.zeros((DPR * s, D), np.float32)
        p1_blk = np.zeros((DPR * s, D), np.float32)
        for i, c in enumerate(doc_order[r * DPR : (r + 1) * DPR]):
            tok = pm[c][pmask[c]]  # [L, D] valid tokens
            L = len(tok)
            if L == 1:
                pa, pb_ = tok.copy(), tok.copy()
            else:
                h = L // 2
                pairs_a = [tok[:h]]
                pairs_b = [tok[h : 2 * h]]
                if L % 2 == 1:
                    pairs_a.append(tok[L - 1 : L])
                    pairs_b.append(tok[0:1])
                pa = np.concatenate(pairs_a, axis=0)
                pb_ = np.concatenate(pairs_b, axis=0)
            npair = len(pa)
            # pad with duplicates of pair 0
            pad = s - npair
            if pad > 0:
                pa = np.concatenate([pa, np.repeat(pa[0:1], pad, axis=0)], axis=0)
                pb_ = np.concatenate([pb_, np.repeat(pb_[0:1], pad, axis=0)], axis=0)
            pd_blk[i * s : (i + 1) * s] = pa - pb_
            p1_blk[i * s : (i + 1) * s] = pb_
        blocks.append(pd_blk.T)
        blocks.append(p1_blk.T)
    pT = np.ascontiguousarray(np.concatenate(blocks, axis=1)).astype(
        ml_dtypes.bfloat16
    )
    return pT


def _prep_in_maps(query_multi, pos_multi, q_mask, p_mask, plan):
    import ml_dtypes

    rows_per_core, nj, doc_order, widths = plan
    qm = np.ascontiguousarray(np.asarray(query_multi, np.float32))
    pm = np.ascontiguousarray(np.asarray(pos_multi, np.float32))
    qmask = np.asarray(q_mask).astype(bool)
    pmask = np.asarray(p_mask).astype(bool)

    pT = _prep_pos(pm, pmask, doc_order, widths)
    ident = np.eye(128, dtype=ml_dtypes.bfloat16)

    in_maps = []
    for c in range(NCORES):
        qtok = np.zeros((nj * 128, D), np.float32)
        qoh = np.zeros((nj * 128, BPC), np.float32)
        pos = 0
        for i, b in enumerate(rows_per_core[c]):
            tok = qm[b][qmask[b]]
            n = len(tok)
            qtok[pos : pos + n] = tok
            qoh[pos : pos + n, i] = 1.0
            pos += n
        qT = np.ascontiguousarray(qtok.T).astype(ml_dtypes.bfloat16)
        qoh3 = np.ascontiguousarray(qoh.reshape(nj, 128, BPC).transpose(1, 0, 2))
        in_maps.append({"pT": pT, "qT": qT, "identity": ident, "qoh": qoh3})
    return in_maps


def _host_losses(dense_sim, S_late):
    """Float64 replica of the reference softmax/CE/KL tail."""

    def softmax_and_logp(z):
        m = z.max(axis=1, keepdims=True)
        e = np.exp(z - m)
        den = e.sum(axis=1, keepdims=True)
        return e / den, (z - m) - np.log(den)

    zd = dense_sim / TAU
    zl = S_late / TAU
    dp, logp_d = softmax_and_logp(zd)
    lp, logp_l = softmax_and_logp(zl)
    idx = np.arange(B)
    single = -logp_d[idx, idx].mean()
    multi = -logp_l[idx, idx].mean()
    kl = (dp * np.log((dp + EPS) / (lp + EPS))).sum(axis=1).mean()
    return single, multi, kl


def run(inputs: dict, trace: bool = False):
    """Run the spmd kernel; returns (loss tuple, BassKernelResults)."""
    qmask = np.asarray(inputs["q_mask"]).astype(bool)
    pmask = np.asarray(inputs["p_mask"]).astype(bool)
    plan = _plan(qmask, pmask)
    rows_per_core, nj, doc_order, widths = plan

    nc = _get_nc(nj, widths)
    in_maps = _prep_in_maps(
        inputs["query_multi"], inputs["pos_multi"], qmask, pmask, plan
    )
    res = run_bass_kernel_spmd(nc, in_maps, core_ids=list(range(NCORES)), trace=trace)

    # Assemble S_raw in original (row, doc) order.
    S_raw = np.zeros((B, B), np.float64)
    inv_doc = np.argsort(doc_order)
    for c in range(NCORES):
        block = np.asarray(res.results[c]["out"], np.float64)  # [BPC, B]
        for i, b in enumerate(rows_per_core[c]):
            S_raw[b] = block[i][inv_doc]

    t_i = np.maximum(qmask.sum(axis=1), 1).astype(np.float64)
    S_late = S_raw / t_i[:, None]

    qs = np.asarray(inputs["query_single"], np.float64)
    ps = np.asarray(inputs["pos_single"], np.float64)
    dense_sim = qs @ ps.T

    single, multi, kl = _host_losses(dense_sim, S_late)
    total = single + multi + kl
    out = (np.float32(total), np.float32(single), np.float32(multi), np.float32(kl))
    return out, res


def kernel(query_single, pos_single, query_multi, pos_multi, q_mask, p_mask):
    out, _ = run(
        {
            "query_single": query_single,
            "pos_single": pos_single,
            "query_multi": query_multi,
            "pos_multi": pos_multi,
            "q_mask": q_mask,
            "p_mask": p_mask,
        }
    )
    return out


# revision 18
# speedup vs baseline: 1.0415x; 1.0189x over previous
"""Trainium2 Bass kernel for nn_JinaPairTraining (dense CE + late-interaction
maxsim CE + KL between the two softmax distributions).

Sharding: data-parallel over the query batch dim Bq. Rows are assigned to the
8 cores to balance valid-q-token counts; every core receives the full
(mask-packed) pos side and computes its rows of the raw maxsim matrix
S_raw[row, doc] = sum_{valid q} max_{valid p} sim.  The host does everything
else: the dense [32,32] logits (tiny), the row softmax / CE / KL in float64,
and the final mean.  Only the O(B^2 T^2 D) sim work runs on device.

Mask packing (exact, no approximation):
  * q side: only valid q tokens are shipped, packed into chunks of 128
    (crossing row boundaries).  The masked one-hot stationary (qoh) of the
    final sum-over-q matmul routes each token slot to its row; pad slots get
    weight 0.
  * p side: only valid pos tokens are shipped.  Tokens are pair-folded
    (max(s0, s1) = s1 + relu(s0 - s1), computed as PE matmuls + one ACT relu
    + an identity-matmul accumulate).  Docs are sorted by pair count and
    grouped into 4 regions of 8 docs; each region pads its docs to the
    region max with duplicate pairs (duplicates never change a max).
  * the kernel is compiled per (chunk-count, region-widths) signature and
    cached; all-ones masks degenerate to the dense full-size layout.
"""

import os
import sys

import numpy as np

for _p in ("/opt/trn_rl_repo",):
    if _p not in sys.path and os.path.isdir(_p):
        sys.path.insert(0, _p)

import concourse.bacc as bacc
import concourse.tile as tile
from concourse import mybir
from concourse.bass_utils import run_bass_kernel_spmd

B, T, D = 32, 256, 128
TAU = 0.02
EPS = 1e-8
NCORES = 8
BPC = B // NCORES  # 4 query rows per core
NREG = 4           # pos regions (8 docs each, sorted by valid-pair count)
DPR = B // NREG    # docs per region

F32 = mybir.dt.float32
BF16 = mybir.dt.bfloat16
AX = mybir.AxisListType
ACT = mybir.ActivationFunctionType


def _build_kernel(nj, widths):
    """nj: q chunks per core; widths: per-region pairs-per-doc (s_r)."""
    nc = bacc.Bacc(None, target_bir_lowering=False, debug=False)

    totw = sum(DPR * s for s in widths)
    pT_d = nc.dram_tensor("pT", [D, 2 * totw], BF16, kind="ExternalInput")
    qT_d = nc.dram_tensor("qT", [D, nj * 128], BF16, kind="ExternalInput")
    ident_d = nc.dram_tensor("identity", [128, 128], BF16, kind="ExternalInput")
    out_d = nc.dram_tensor("out", [128, NREG, nj, DPR], F32, kind="ExternalOutput")

    roff = np.cumsum([0] + [2 * DPR * s for s in widths]).tolist()

    with tile.TileContext(nc) as tc:
        with tc.tile_pool(name="sb", bufs=1) as sb:
            # qT + smalls ride the ACT queue; p regions stream on the SP
            # queue in parallel.
            qT = sb.tile([D, nj * 128], BF16)
            nc.scalar.dma_start(out=qT, in_=qT_d[:, :])
            pT = sb.tile([D, 2 * totw], BF16)
            for r in range(NREG):
                eng = nc.sync if r % 2 == 0 else nc.scalar
                eng.dma_start(
                    out=pT[:, roff[r] : roff[r + 1]],
                    in_=pT_d[:, roff[r] : roff[r + 1]],
                )
            ident = sb.tile([128, 128], BF16)
            nc.sync.dma_start(out=ident, in_=ident_d[:, :])

            # mx[q, r, j, i]: per q chunk j, per pos doc i of region r (docs
            # in sorted order), the masked max over that doc's tokens.  The
            # region-r slab is contiguous so it can be DMAed out as soon as
            # region r finishes; the host does the masked sum over q.
            mx = sb.tile([128, NREG, nj, DPR], F32)

            with (
                tc.tile_pool(name="pb", bufs=2, space="PSUM") as pb,
                tc.tile_pool(name="rp", bufs=2) as rp,
            ):
                for r in range(NREG):
                    w = DPR * widths[r]
                    pd = pT[:, roff[r] : roff[r] + w]
                    p1 = pT[:, roff[r] + w : roff[r + 1]]
                    nchunk = (w + 511) // 512
                    for j in range(nj):
                        qj = qT[:, j * 128 : (j + 1) * 128]
                        ps_d = pb.tile([128, w], F32, name="ps_d")
                        for k in range(nchunk):
                            sl = slice(k * 512, min((k + 1) * 512, w))
                            nc.tensor.matmul(
                                ps_d[:, sl], qj, pd[:, sl], start=True, stop=True
                            )
                        # ps_m as two 4-doc tiles (each <= 1 PSUM bank) so
                        # each is freed right after its own reduce.
                        hw_ = w // 2
                        ps_ms = [
                            pb.tile([128, hw_], F32, name=f"ps_m{h}")
                            for h in range(2)
                        ]
                        for h in range(2):
                            nc.tensor.matmul(
                                ps_ms[h],
                                qj,
                                p1[:, h * hw_ : (h + 1) * hw_],
                                start=True,
                                stop=False,
                            )
                        relu_sb = rp.tile([128, w], BF16, name="relu_sb")
                        nc.scalar.activation(relu_sb, ps_d, ACT.Relu)
                        for h in range(2):
                            nc.tensor.matmul(
                                ps_ms[h],
                                ident,
                                relu_sb[:, h * hw_ : (h + 1) * hw_],
                                start=False,
                                stop=True,
                            )
                        for h in range(2):
                            nc.vector.reduce_max(
                                out=mx[
                                    :,
                                    r,
                                    j,
                                    h * (DPR // 2) : (h + 1) * (DPR // 2),
                                ],
                                in_=ps_ms[h].rearrange(
                                    "p (g s) -> p g s", s=widths[r]
                                ),
                                axis=AX.X,
                            )
                    # Ship this region's mx slab while later regions compute.
                    nc.sync.dma_start(out=out_d[:, r], in_=mx[:, r])

    nc.compile()
    return nc


_NC_CACHE = {}
_LAST_NC = None


def _get_nc(nj=None, widths=None):
    global _LAST_NC
    if nj is None:
        return _LAST_NC
    key = (nj, tuple(widths))
    if key not in _NC_CACHE:
        _NC_CACHE[key] = _build_kernel(nj, widths)
    _LAST_NC = _NC_CACHE[key]
    return _LAST_NC


def _pad4(x):
    return (x + 3) & ~3


def _plan(q_mask, p_mask):
    """Row->core assignment, q chunk count, pos doc order + region widths."""
    qlen = q_mask.sum(axis=1).astype(int)
    # Balance valid-q counts across cores (4 rows each): greedy LPT, then
    # pairwise-swap refinement to minimize the max core sum (which sets the
    # compiled chunk count for every core).
    order = np.argsort(-qlen, kind="stable")
    sums = [0] * NCORES
    counts = [0] * NCORES
    rows_per_core = [[] for _ in range(NCORES)]
    for b in order:
        cands = [c for c in range(NCORES) if counts[c] < BPC]
        c = min(cands, key=lambda c: sums[c])
        rows_per_core[c].append(int(b))
        sums[c] += int(qlen[b])
        counts[c] += 1
    improved = True
    while improved:
        improved = False
        hi = int(np.argmax(sums))
        for lo in sorted(range(NCORES), key=lambda c: sums[c]):
            if lo == hi:
                continue
            for i, bh in enumerate(rows_per_core[hi]):
                for k, bl in enumerate(rows_per_core[lo]):
                    delta = int(qlen[bh]) - int(qlen[bl])
                    if delta <= 0:
                        continue
                    new_hi = sums[hi] - delta
                    new_lo = sums[lo] + delta
                    if max(new_hi, new_lo) < sums[hi]:
                        rows_per_core[hi][i], rows_per_core[lo][k] = bl, bh
                        sums[hi], sums[lo] = new_hi, new_lo
                        improved = True
                        break
                if improved:
                    break
            if improved:
                break
    nj = max(1, (max(sums) + 127) // 128)

    # Pos docs sorted by valid-pair count, 4 regions of 8.
    plen = p_mask.sum(axis=1).astype(int)
    pairs = (plen + 1) // 2
    doc_order = np.argsort(pairs, kind="stable")
    widths = []
    for r in range(NREG):
        grp = doc_order[r * DPR : (r + 1) * DPR]
        widths.append(int(_pad4(max(1, pairs[grp].max()))))
    return rows_per_core, nj, doc_order, widths


def _prep_pos(pm, pmask, doc_order, widths):
    """Packed [D, 2*totw] bf16 pos tensor: per region [pd block | p1 block]."""
    import ml_dtypes

    blocks = []
    for r in range(NREG):
        s = widths[r]
        pd_blk = np.zeros((DPR * s, D), np.float32)
        p1_blk = np.zeros((DPR * s, D), np.float32)
        for i, c in enumerate(doc_order[r * DPR : (r + 1) * DPR]):
            tok = pm[c][pmask[c]]  # [L, D] valid tokens
            L = len(tok)
            if L == 1:
                pa, pb_ = tok.copy(), tok.copy()
            else:
                h = L // 2
                pairs_a = [tok[:h]]
                pairs_b = [tok[h : 2 * h]]
                if L % 2 == 1:
                    pairs_a.append(tok[L - 1 : L])
                    pairs_b.append(tok[0:1])
                pa = np.concatenate(pairs_a, axis=0)
                pb_ = np.concatenate(pairs_b, axis=0)
            npair = len(pa)
            # pad with duplicates of pair 0
            pad = s - npair
            if pad > 0:
                pa = np.concatenate([pa, np.repeat(pa[0:1], pad, axis=0)], axis=0)
                pb_ = np.concatenate([pb_, np.repeat(pb_[0:1], pad, axis=0)], axis=0)
            pd_blk[i * s : (i + 1) * s] = pa - pb_
            p1_blk[i * s : (i + 1) * s] = pb_
        blocks.append(pd_blk.T)
        blocks.append(p1_blk.T)
    pT = np.ascontiguousarray(np.concatenate(blocks, axis=1)).astype(
        ml_dtypes.bfloat16
    )
    return pT


def _prep_in_maps(query_multi, pos_multi, q_mask, p_mask, plan):
    import ml_dtypes

    rows_per_core, nj, doc_order, widths = plan
    qm = np.ascontiguousarray(np.asarray(query_multi, np.float32))
    pm = np.ascontiguousarray(np.asarray(pos_multi, np.float32))
    qmask = np.asarray(q_mask).astype(bool)
    pmask = np.asarray(p_mask).astype(bool)

    pT = _prep_pos(pm, pmask, doc_order, widths)
    ident = np.eye(128, dtype=ml_dtypes.bfloat16)

    in_maps = []
    qohs = []
    for c in range(NCORES):
        qtok = np.zeros((nj * 128, D), np.float32)
        qoh = np.zeros((nj * 128, BPC), np.float32)
        pos = 0
        for i, b in enumerate(rows_per_core[c]):
            tok = qm[b][qmask[b]]
            n = len(tok)
            qtok[pos : pos + n] = tok
            qoh[pos : pos + n, i] = 1.0
            pos += n
        qT = np.ascontiguousarray(qtok.T).astype(ml_dtypes.bfloat16)
        in_maps.append({"pT": pT, "qT": qT, "identity": ident})
        qohs.append(qoh)  # [nj*128, BPC] host-side sum weights
    return in_maps, qohs


def _host_losses(dense_sim, S_late):
    """Float64 replica of the reference softmax/CE/KL tail."""

    def softmax_and_logp(z):
        m = z.max(axis=1, keepdims=True)
        e = np.exp(z - m)
        den = e.sum(axis=1, keepdims=True)
        return e / den, (z - m) - np.log(den)

    zd = dense_sim / TAU
    zl = S_late / TAU
    dp, logp_d = softmax_and_logp(zd)
    lp, logp_l = softmax_and_logp(zl)
    idx = np.arange(B)
    single = -logp_d[idx, idx].mean()
    multi = -logp_l[idx, idx].mean()
    kl = (dp * np.log((dp + EPS) / (lp + EPS))).sum(axis=1).mean()
    return single, multi, kl


def run(inputs: dict, trace: bool = False):
    """Run the spmd kernel; returns (loss tuple, BassKernelResults)."""
    qmask = np.asarray(inputs["q_mask"]).astype(bool)
    pmask = np.asarray(inputs["p_mask"]).astype(bool)
    plan = _plan(qmask, pmask)
    rows_per_core, nj, doc_order, widths = plan

    nc = _get_nc(nj, widths)
    in_maps, qohs = _prep_in_maps(
        inputs["query_multi"], inputs["pos_multi"], qmask, pmask, plan
    )
    res = run_bass_kernel_spmd(nc, in_maps, core_ids=list(range(NCORES)), trace=trace)

    # Assemble S_raw in original (row, doc) order.  Device output is
    # mx[slot_in_chunk, region, chunk, doc_in_region]; the masked sum over q
    # slots is a tiny host einsum.
    S_raw = np.zeros((B, B), np.float64)
    for c in range(NCORES):
        mx = np.asarray(res.results[c]["out"], np.float64)  # [128, NREG, nj, DPR]
        mx2 = mx.transpose(2, 0, 1, 3).reshape(nj * 128, B)  # [slot, sorted doc]
        block = qohs[c].T @ mx2  # [BPC, B]
        for i, b in enumerate(rows_per_core[c]):
            S_raw[b, doc_order] = block[i]

    t_i = np.maximum(qmask.sum(axis=1), 1).astype(np.float64)
    S_late = S_raw / t_i[:, None]

    qs = np.asarray(inputs["query_single"], np.float64)
    ps = np.asarray(inputs["pos_single"], np.float64)
    dense_sim = qs @ ps.T

    single, multi, kl = _host_losses(dense_sim, S_late)
    total = single + multi + kl
    out = (np.float32(total), np.float32(single), np.float32(multi), np.float32(kl))
    return out, res


def kernel(query_single, pos_single, query_multi, pos_multi, q_mask, p_mask):
    out, _ = run(
        {
            "query_single": query_single,
            "pos_single": pos_single,
            "query_multi": query_multi,
            "pos_multi": pos_multi,
            "q_mask": q_mask,
            "p_mask": p_mask,
        }
    )
    return out


# revision 21
# speedup vs baseline: 1.0828x; 1.0397x over previous
"""Trainium2 Bass kernel for nn_JinaPairTraining (dense CE + late-interaction
maxsim CE + KL between the two softmax distributions).

Sharding: data-parallel over the query batch dim Bq. Rows are assigned to the
8 cores to balance valid-q-token counts; every core receives the full
(mask-packed) pos side and computes its rows of the raw maxsim matrix
S_raw[row, doc] = sum_{valid q} max_{valid p} sim.  The host does everything
else: the dense [32,32] logits (tiny), the row softmax / CE / KL in float64,
and the final mean.  Only the O(B^2 T^2 D) sim work runs on device.

Mask packing (exact, no approximation):
  * q side: only valid q tokens are shipped, packed into chunks of 128
    (crossing row boundaries).  The masked one-hot stationary (qoh) of the
    final sum-over-q matmul routes each token slot to its row; pad slots get
    weight 0.
  * p side: only valid pos tokens are shipped.  Tokens are pair-folded
    (max(s0, s1) = s1 + relu(s0 - s1), computed as PE matmuls + one ACT relu
    + an identity-matmul accumulate).  Docs are sorted by pair count and
    grouped into 4 regions of 8 docs; each region pads its docs to the
    region max with duplicate pairs (duplicates never change a max).
  * the kernel is compiled per (chunk-count, region-widths) signature and
    cached; all-ones masks degenerate to the dense full-size layout.
"""

import os
import sys

import numpy as np

for _p in ("/opt/trn_rl_repo",):
    if _p not in sys.path and os.path.isdir(_p):
        sys.path.insert(0, _p)

import concourse.bacc as bacc
import concourse.tile as tile
from concourse import mybir
from concourse.bass_utils import run_bass_kernel_spmd

B, T, D = 32, 256, 128
TAU = 0.02
EPS = 1e-8
NCORES = 8
BPC = B // NCORES  # 4 query rows per core
NREG = 4           # pos regions (8 docs each, sorted by valid-pair count)
DPR = B // NREG    # docs per region

F32 = mybir.dt.float32
BF16 = mybir.dt.bfloat16
AX = mybir.AxisListType
ACT = mybir.ActivationFunctionType


def _build_kernel(nj, widths):
    """nj: q chunks per core; widths: per-region pairs-per-doc (s_r)."""
    nc = bacc.Bacc(None, target_bir_lowering=False, debug=False)

    totw = sum(DPR * s for s in widths)
    pT_d = nc.dram_tensor("pT", [D, 2 * totw], BF16, kind="ExternalInput")
    qT_d = nc.dram_tensor("qT", [D, nj * 128], BF16, kind="ExternalInput")
    ident_d = nc.dram_tensor("identity", [128, 128], BF16, kind="ExternalInput")
    out_d = nc.dram_tensor("out", [128, NREG, nj, DPR], F32, kind="ExternalOutput")

    roff = np.cumsum([0] + [2 * DPR * s for s in widths]).tolist()

    with tile.TileContext(nc) as tc:
        with tc.tile_pool(name="sb", bufs=1) as sb:
            # PE warm-up: dummy matmuls during the input-DMA shadow so the
            # p-state ramp (HAM) is done before the first real matmul.
            with tc.tile_pool(name="warm", bufs=1, space="PSUM") as wp:
                wsrc = sb.tile([128, 512], BF16)
                nc.vector.memset(wsrc, 0.0)
                wdst = wp.tile([128, 512], F32)
                for _ in range(6):
                    nc.tensor.matmul(wdst, wsrc[:, :128], wsrc, start=True, stop=True)
            # qT + smalls ride the ACT queue; p regions stream on the SP
            # queue in parallel.
            qT = sb.tile([D, nj * 128], BF16)
            nc.scalar.dma_start(out=qT, in_=qT_d[:, :])
            pT = sb.tile([D, 2 * totw], BF16)
            for r in range(NREG):
                eng = nc.sync if r % 2 == 0 else nc.scalar
                eng.dma_start(
                    out=pT[:, roff[r] : roff[r + 1]],
                    in_=pT_d[:, roff[r] : roff[r + 1]],
                )
            ident = sb.tile([128, 128], BF16)
            nc.sync.dma_start(out=ident, in_=ident_d[:, :])

            # mx[q, r, j, i]: per q chunk j, per pos doc i of region r (docs
            # in sorted order), the masked max over that doc's tokens.  The
            # region-r slab is contiguous so it can be DMAed out as soon as
            # region r finishes; the host does the masked sum over q.
            mx = sb.tile([128, NREG, nj, DPR], F32)

            with (
                tc.tile_pool(name="pb", bufs=2, space="PSUM") as pb,
                tc.tile_pool(name="rp", bufs=2) as rp,
            ):
                for r in range(NREG):
                    w = DPR * widths[r]
                    pd = pT[:, roff[r] : roff[r] + w]
                    p1 = pT[:, roff[r] + w : roff[r + 1]]
                    nchunk = (w + 511) // 512
                    for j in range(nj):
                        qj = qT[:, j * 128 : (j + 1) * 128]
                        ps_d = pb.tile([128, w], F32, name="ps_d")
                        for k in range(nchunk):
                            sl = slice(k * 512, min((k + 1) * 512, w))
                            nc.tensor.matmul(
                                ps_d[:, sl], qj, pd[:, sl], start=True, stop=True
                            )
                        # Wide regions: ps_m as two 4-doc tiles (each <= 1
                        # PSUM bank) so each is freed right after its own
                        # (short) reduce.  Narrow regions: one tile + one
                        # reduce — the hold is short and the saved reduce
                        # init wins.
                        nsplit = 2 if w > 512 else 1
                        hw_ = w // nsplit
                        gpr = DPR // nsplit
                        ps_ms = [
                            pb.tile([128, hw_], F32, name=f"ps_m{h}")
                            for h in range(nsplit)
                        ]
                        for h in range(nsplit):
                            for k in range(0, hw_, 512):
                                sl = slice(k, min(k + 512, hw_))
                                nc.tensor.matmul(
                                    ps_ms[h][:, sl],
                                    qj,
                                    p1[:, h * hw_ + sl.start : h * hw_ + sl.stop],
                                    start=True,
                                    stop=False,
                                )
                        relu_sb = rp.tile([128, w], BF16, name="relu_sb")
                        nc.scalar.activation(relu_sb, ps_d, ACT.Relu)
                        for h in range(nsplit):
                            for k in range(0, hw_, 512):
                                sl = slice(k, min(k + 512, hw_))
                                nc.tensor.matmul(
                                    ps_ms[h][:, sl],
                                    ident,
                                    relu_sb[:, h * hw_ + sl.start : h * hw_ + sl.stop],
                                    start=False,
                                    stop=True,
                                )
                        for h in range(nsplit):
                            nc.vector.reduce_max(
                                out=mx[:, r, j, h * gpr : (h + 1) * gpr],
                                in_=ps_ms[h].rearrange(
                                    "p (g s) -> p g s", s=widths[r]
                                ),
                                axis=AX.X,
                            )
                    # Ship this region's mx slab while later regions compute.
                    nc.sync.dma_start(out=out_d[:, r], in_=mx[:, r])

    nc.compile()
    return nc


_NC_CACHE = {}
_LAST_NC = None


def _get_nc(nj=None, widths=None):
    global _LAST_NC
    if nj is None:
        return _LAST_NC
    key = (nj, tuple(widths))
    if key not in _NC_CACHE:
        _NC_CACHE[key] = _build_kernel(nj, widths)
    _LAST_NC = _NC_CACHE[key]
    return _LAST_NC


def _pad4(x):
    return (x + 3) & ~3


def _plan(q_mask, p_mask):
    """Row->core assignment, q chunk count, pos doc order + region widths."""
    qlen = q_mask.sum(axis=1).astype(int)
    # Balance valid-q counts across cores (4 rows each): greedy LPT, then
    # pairwise-swap refinement to minimize the max core sum (which sets the
    # compiled chunk count for every core).
    order = np.argsort(-qlen, kind="stable")
    sums = [0] * NCORES
    counts = [0] * NCORES
    rows_per_core = [[] for _ in range(NCORES)]
    for b in order:
        cands = [c for c in range(NCORES) if counts[c] < BPC]
        c = min(cands, key=lambda c: sums[c])
        rows_per_core[c].append(int(b))
        sums[c] += int(qlen[b])
        counts[c] += 1
    improved = True
    while improved:
        improved = False
        hi = int(np.argmax(sums))
        for lo in sorted(range(NCORES), key=lambda c: sums[c]):
            if lo == hi:
                continue
            for i, bh in enumerate(rows_per_core[hi]):
                for k, bl in enumerate(rows_per_core[lo]):
                    delta = int(qlen[bh]) - int(qlen[bl])
                    if delta <= 0:
                        continue
                    new_hi = sums[hi] - delta
                    new_lo = sums[lo] + delta
                    if max(new_hi, new_lo) < sums[hi]:
                        rows_per_core[hi][i], rows_per_core[lo][k] = bl, bh
                        sums[hi], sums[lo] = new_hi, new_lo
                        improved = True
                        break
                if improved:
                    break
            if improved:
                break
    nj = max(1, (max(sums) + 127) // 128)

    # Pos docs sorted by valid-pair count, 4 regions of 8.
    plen = p_mask.sum(axis=1).astype(int)
    pairs = (plen + 1) // 2
    doc_order = np.argsort(pairs, kind="stable")
    widths = []
    for r in range(NREG):
        grp = doc_order[r * DPR : (r + 1) * DPR]
        widths.append(int(_pad4(max(1, pairs[grp].max()))))
    return rows_per_core, nj, doc_order, widths


def _prep_pos(pm, pmask, doc_order, widths):
    """Packed [D, 2*totw] bf16 pos tensor: per region [pd block | p1 block]."""
    import ml_dtypes

    blocks = []
    for r in range(NREG):
        s = widths[r]
        pd_blk = np.zeros((DPR * s, D), np.float32)
        p1_blk = np.zeros((DPR * s, D), np.float32)
        for i, c in enumerate(doc_order[r * DPR : (r + 1) * DPR]):
            tok = pm[c][pmask[c]]  # [L, D] valid tokens
            L = len(tok)
            if L == 1:
                pa, pb_ = tok.copy(), tok.copy()
            else:
                h = L // 2
                pairs_a = [tok[:h]]
                pairs_b = [tok[h : 2 * h]]
                if L % 2 == 1:
                    pairs_a.append(tok[L - 1 : L])
                    pairs_b.append(tok[0:1])
                pa = np.concatenate(pairs_a, axis=0)
                pb_ = np.concatenate(pairs_b, axis=0)
            npair = len(pa)
            # pad with duplicates of pair 0
            pad = s - npair
            if pad > 0:
                pa = np.concatenate([pa, np.repeat(pa[0:1], pad, axis=0)], axis=0)
                pb_ = np.concatenate([pb_, np.repeat(pb_[0:1], pad, axis=0)], axis=0)
            pd_blk[i * s : (i + 1) * s] = pa - pb_
            p1_blk[i * s : (i + 1) * s] = pb_
        blocks.append(pd_blk.T)
        blocks.append(p1_blk.T)
    pT = np.ascontiguousarray(np.concatenate(blocks, axis=1)).astype(
        ml_dtypes.bfloat16
    )
    return pT


def _prep_in_maps(query_multi, pos_multi, q_mask, p_mask, plan):
    import ml_dtypes

    rows_per_core, nj, doc_order, widths = plan
    qm = np.ascontiguousarray(np.asarray(query_multi, np.float32))
    pm = np.ascontiguousarray(np.asarray(pos_multi, np.float32))
    qmask = np.asarray(q_mask).astype(bool)
    pmask = np.asarray(p_mask).astype(bool)

    pT = _prep_pos(pm, pmask, doc_order, widths)
    ident = np.eye(128, dtype=ml_dtypes.bfloat16)

    in_maps = []
    qohs = []
    for c in range(NCORES):
        qtok = np.zeros((nj * 128, D), np.float32)
        qoh = np.zeros((nj * 128, BPC), np.float32)
        pos = 0
        for i, b in enumerate(rows_per_core[c]):
            tok = qm[b][qmask[b]]
            n = len(tok)
            qtok[pos : pos + n] = tok
            qoh[pos : pos + n, i] = 1.0
            pos += n
        qT = np.ascontiguousarray(qtok.T).astype(ml_dtypes.bfloat16)
        in_maps.append({"pT": pT, "qT": qT, "identity": ident})
        qohs.append(qoh)  # [nj*128, BPC] host-side sum weights
    return in_maps, qohs


def _host_losses(dense_sim, S_late):
    """Float64 replica of the reference softmax/CE/KL tail."""

    def softmax_and_logp(z):
        m = z.max(axis=1, keepdims=True)
        e = np.exp(z - m)
        den = e.sum(axis=1, keepdims=True)
        return e / den, (z - m) - np.log(den)

    zd = dense_sim / TAU
    zl = S_late / TAU
    dp, logp_d = softmax_and_logp(zd)
    lp, logp_l = softmax_and_logp(zl)
    idx = np.arange(B)
    single = -logp_d[idx, idx].mean()
    multi = -logp_l[idx, idx].mean()
    kl = (dp * np.log((dp + EPS) / (lp + EPS))).sum(axis=1).mean()
    return single, multi, kl


def run(inputs: dict, trace: bool = False):
    """Run the spmd kernel; returns (loss tuple, BassKernelResults)."""
    qmask = np.asarray(inputs["q_mask"]).astype(bool)
    pmask = np.asarray(inputs["p_mask"]).astype(bool)
    plan = _plan(qmask, pmask)
    rows_per_core, nj, doc_order, widths = plan

    nc = _get_nc(nj, widths)
    in_maps, qohs = _prep_in_maps(
        inputs["query_multi"], inputs["pos_multi"], qmask, pmask, plan
    )
    res = run_bass_kernel_spmd(nc, in_maps, core_ids=list(range(NCORES)), trace=trace)

    # Assemble S_raw in original (row, doc) order.  Device output is
    # mx[slot_in_chunk, region, chunk, doc_in_region]; the masked sum over q
    # slots is a tiny host einsum.
    S_raw = np.zeros((B, B), np.float64)
    for c in range(NCORES):
        mx = np.asarray(res.results[c]["out"], np.float64)  # [128, NREG, nj, DPR]
        mx2 = mx.transpose(2, 0, 1, 3).reshape(nj * 128, B)  # [slot, sorted doc]
        block = qohs[c].T @ mx2  # [BPC, B]
        for i, b in enumerate(rows_per_core[c]):
            S_raw[b, doc_order] = block[i]

    t_i = np.maximum(qmask.sum(axis=1), 1).astype(np.float64)
    S_late = S_raw / t_i[:, None]

    qs = np.asarray(inputs["query_single"], np.float64)
    ps = np.asarray(inputs["pos_single"], np.float64)
    dense_sim = qs @ ps.T

    single, multi, kl = _host_losses(dense_sim, S_late)
    total = single + multi + kl
    out = (np.float32(total), np.float32(single), np.float32(multi), np.float32(kl))
    return out, res


def kernel(query_single, pos_single, query_multi, pos_multi, q_mask, p_mask):
    out, _ = run(
        {
            "query_single": query_single,
            "pos_single": pos_single,
            "query_multi": query_multi,
            "pos_multi": pos_multi,
            "q_mask": q_mask,
            "p_mask": p_mask,
        }
    )
    return out


# revision 23
# speedup vs baseline: 1.0998x; 1.0157x over previous
"""Trainium2 Bass kernel for nn_JinaPairTraining (dense CE + late-interaction
maxsim CE + KL between the two softmax distributions).

Sharding: data-parallel over the query batch dim Bq. Rows are assigned to the
8 cores to balance valid-q-token counts; every core receives the full
(mask-packed) pos side and computes its rows of the raw maxsim matrix
S_raw[row, doc] = sum_{valid q} max_{valid p} sim.  The host does everything
else: the dense [32,32] logits (tiny), the row softmax / CE / KL in float64,
and the final mean.  Only the O(B^2 T^2 D) sim work runs on device.

Mask packing (exact, no approximation):
  * q side: only valid q tokens are shipped, packed into chunks of 128
    (crossing row boundaries).  The masked one-hot stationary (qoh) of the
    final sum-over-q matmul routes each token slot to its row; pad slots get
    weight 0.
  * p side: only valid pos tokens are shipped.  Tokens are pair-folded
    (max(s0, s1) = s1 + relu(s0 - s1), computed as PE matmuls + one ACT relu
    + an identity-matmul accumulate).  Docs are sorted by pair count and
    grouped into 4 regions of 8 docs; each region pads its docs to the
    region max with duplicate pairs (duplicates never change a max).
  * the kernel is compiled per (chunk-count, region-widths) signature and
    cached; all-ones masks degenerate to the dense full-size layout.
"""

import os
import sys

import numpy as np

for _p in ("/opt/trn_rl_repo",):
    if _p not in sys.path and os.path.isdir(_p):
        sys.path.insert(0, _p)

import concourse.bacc as bacc
import concourse.tile as tile
from concourse import mybir
from concourse.bass_utils import run_bass_kernel_spmd

B, T, D = 32, 256, 128
TAU = 0.02
EPS = 1e-8
NCORES = 8
BPC = B // NCORES  # 4 query rows per core
NREG = 4           # pos regions (8 docs each, sorted by valid-pair count)
DPR = B // NREG    # docs per region

F32 = mybir.dt.float32
BF16 = mybir.dt.bfloat16
AX = mybir.AxisListType
ACT = mybir.ActivationFunctionType


def _build_kernel(nj, widths):
    """nj: q chunks per core; widths: per-region pairs-per-doc (s_r)."""
    nc = bacc.Bacc(None, target_bir_lowering=False, debug=False)

    totw = sum(DPR * s for s in widths)
    pT_d = nc.dram_tensor("pT", [D, 2 * totw], BF16, kind="ExternalInput")
    qT_d = nc.dram_tensor("qT", [D, nj * 128], BF16, kind="ExternalInput")
    ident_d = nc.dram_tensor("identity", [128, 128], BF16, kind="ExternalInput")
    out_d = nc.dram_tensor("out", [128, NREG, nj, DPR], F32, kind="ExternalOutput")

    roff = np.cumsum([0] + [2 * DPR * s for s in widths]).tolist()

    with tile.TileContext(nc) as tc:
        with tc.tile_pool(name="sb", bufs=1) as sb:
            # PE warm-up: dummy matmuls during the input-DMA shadow so the
            # p-state ramp (HAM) is done before the first real matmul.
            with tc.tile_pool(name="warm", bufs=1, space="PSUM") as wp:
                wsrc = sb.tile([128, 512], BF16)
                nc.vector.memset(wsrc, 0.0)
                wdst = wp.tile([128, 512], F32)
                for _ in range(6):
                    nc.tensor.matmul(wdst, wsrc[:, :128], wsrc, start=True, stop=True)
            # qT + smalls ride the ACT queue; p regions stream on the SP
            # queue in parallel.
            ident = sb.tile([128, 128], BF16)
            nc.sync.dma_start(out=ident, in_=ident_d[:, :])
            qT = sb.tile([D, nj * 128], BF16)
            nc.scalar.dma_start(out=qT, in_=qT_d[:, :])
            pT = sb.tile([D, 2 * totw], BF16)
            for r in range(NREG):
                eng = nc.sync if r % 2 == 0 else nc.scalar
                eng.dma_start(
                    out=pT[:, roff[r] : roff[r + 1]],
                    in_=pT_d[:, roff[r] : roff[r + 1]],
                )

            # mx[q, r, j, i]: per q chunk j, per pos doc i of region r (docs
            # in sorted order), the masked max over that doc's tokens.  The
            # region-r slab is contiguous so it can be DMAed out as soon as
            # region r finishes; the host does the masked sum over q.
            mx = sb.tile([128, NREG, nj, DPR], F32)

            with (
                tc.tile_pool(name="pb", bufs=2, space="PSUM") as pb,
                tc.tile_pool(name="rp", bufs=2) as rp,
            ):
                for r in range(NREG):
                    w = DPR * widths[r]
                    pd = pT[:, roff[r] : roff[r] + w]
                    p1 = pT[:, roff[r] + w : roff[r + 1]]
                    nchunk = (w + 511) // 512
                    for j in range(nj):
                        qj = qT[:, j * 128 : (j + 1) * 128]
                        ps_d = pb.tile([128, w], F32, name="ps_d")
                        for k in range(nchunk):
                            sl = slice(k * 512, min((k + 1) * 512, w))
                            nc.tensor.matmul(
                                ps_d[:, sl], qj, pd[:, sl], start=True, stop=True
                            )
                        # Wide regions: ps_m as two 4-doc tiles (each <= 1
                        # PSUM bank) so each is freed right after its own
                        # (short) reduce.  Narrow regions: one tile + one
                        # reduce — the hold is short and the saved reduce
                        # init wins.
                        nsplit = 2 if w > 512 else 1
                        hw_ = w // nsplit
                        gpr = DPR // nsplit
                        ps_ms = [
                            pb.tile([128, hw_], F32, name=f"ps_m{h}")
                            for h in range(nsplit)
                        ]
                        for h in range(nsplit):
                            for k in range(0, hw_, 512):
                                sl = slice(k, min(k + 512, hw_))
                                nc.tensor.matmul(
                                    ps_ms[h][:, sl],
                                    qj,
                                    p1[:, h * hw_ + sl.start : h * hw_ + sl.stop],
                                    start=True,
                                    stop=False,
                                )
                        relu_sb = rp.tile([128, w], BF16, name="relu_sb")
                        nc.scalar.activation(relu_sb, ps_d, ACT.Relu)
                        for h in range(nsplit):
                            for k in range(0, hw_, 512):
                                sl = slice(k, min(k + 512, hw_))
                                nc.tensor.matmul(
                                    ps_ms[h][:, sl],
                                    ident,
                                    relu_sb[:, h * hw_ + sl.start : h * hw_ + sl.stop],
                                    start=False,
                                    stop=True,
                                )
                        for h in range(nsplit):
                            nc.vector.reduce_max(
                                out=mx[:, r, j, h * gpr : (h + 1) * gpr],
                                in_=ps_ms[h].rearrange(
                                    "p (g s) -> p g s", s=widths[r]
                                ),
                                axis=AX.X,
                            )
                    # Ship this region's mx slab while later regions compute.
                    # The last region goes out per-chunk so the final DMA is
                    # tiny and starts right at the last reduce.
                    if r < NREG - 1:
                        nc.sync.dma_start(out=out_d[:, r], in_=mx[:, r])
                    else:
                        for jj in range(nj):
                            nc.sync.dma_start(
                                out=out_d[:, r, jj], in_=mx[:, r, jj]
                            )

    nc.compile()
    return nc


_NC_CACHE = {}
_LAST_NC = None


def _get_nc(nj=None, widths=None):
    global _LAST_NC
    if nj is None:
        return _LAST_NC
    key = (nj, tuple(widths))
    if key not in _NC_CACHE:
        _NC_CACHE[key] = _build_kernel(nj, widths)
    _LAST_NC = _NC_CACHE[key]
    return _LAST_NC


def _pad4(x):
    return (x + 3) & ~3


def _plan(q_mask, p_mask):
    """Row->core assignment, q chunk count, pos doc order + region widths."""
    qlen = q_mask.sum(axis=1).astype(int)
    # Balance valid-q counts across cores (4 rows each): greedy LPT, then
    # pairwise-swap refinement to minimize the max core sum (which sets the
    # compiled chunk count for every core).
    order = np.argsort(-qlen, kind="stable")
    sums = [0] * NCORES
    counts = [0] * NCORES
    rows_per_core = [[] for _ in range(NCORES)]
    for b in order:
        cands = [c for c in range(NCORES) if counts[c] < BPC]
        c = min(cands, key=lambda c: sums[c])
        rows_per_core[c].append(int(b))
        sums[c] += int(qlen[b])
        counts[c] += 1
    improved = True
    while improved:
        improved = False
        hi = int(np.argmax(sums))
        for lo in sorted(range(NCORES), key=lambda c: sums[c]):
            if lo == hi:
                continue
            for i, bh in enumerate(rows_per_core[hi]):
                for k, bl in enumerate(rows_per_core[lo]):
                    delta = int(qlen[bh]) - int(qlen[bl])
                    if delta <= 0:
                        continue
                    new_hi = sums[hi] - delta
                    new_lo = sums[lo] + delta
                    if max(new_hi, new_lo) < sums[hi]:
                        rows_per_core[hi][i], rows_per_core[lo][k] = bl, bh
                        sums[hi], sums[lo] = new_hi, new_lo
                        improved = True
                        break
                if improved:
                    break
            if improved:
                break
    nj = max(1, (max(sums) + 127) // 128)

    # Pos docs sorted by valid-pair count, 4 regions of 8.
    plen = p_mask.sum(axis=1).astype(int)
    pairs = (plen + 1) // 2
    doc_order = np.argsort(pairs, kind="stable")
    widths = []
    for r in range(NREG):
        grp = doc_order[r * DPR : (r + 1) * DPR]
        widths.append(int(_pad4(max(1, pairs[grp].max()))))
    return rows_per_core, nj, doc_order, widths


def _prep_pos(pm, pmask, doc_order, widths):
    """Packed [D, 2*totw] bf16 pos tensor: per region [pd block | p1 block]."""
    import ml_dtypes

    blocks = []
    for r in range(NREG):
        s = widths[r]
        pd_blk = np.zeros((DPR * s, D), np.float32)
        p1_blk = np.zeros((DPR * s, D), np.float32)
        for i, c in enumerate(doc_order[r * DPR : (r + 1) * DPR]):
            tok = pm[c][pmask[c]]  # [L, D] valid tokens
            L = len(tok)
            if L == 1:
                pa, pb_ = tok.copy(), tok.copy()
            else:
                h = L // 2
                pairs_a = [tok[:h]]
                pairs_b = [tok[h : 2 * h]]
                if L % 2 == 1:
                    pairs_a.append(tok[L - 1 : L])
                    pairs_b.append(tok[0:1])
                pa = np.concatenate(pairs_a, axis=0)
                pb_ = np.concatenate(pairs_b, axis=0)
            npair = len(pa)
            # pad with duplicates of pair 0
            pad = s - npair
            if pad > 0:
                pa = np.concatenate([pa, np.repeat(pa[0:1], pad, axis=0)], axis=0)
                pb_ = np.concatenate([pb_, np.repeat(pb_[0:1], pad, axis=0)], axis=0)
            pd_blk[i * s : (i + 1) * s] = pa - pb_
            p1_blk[i * s : (i + 1) * s] = pb_
        blocks.append(pd_blk.T)
        blocks.append(p1_blk.T)
    pT = np.ascontiguousarray(np.concatenate(blocks, axis=1)).astype(
        ml_dtypes.bfloat16
    )
    return pT


def _prep_in_maps(query_multi, pos_multi, q_mask, p_mask, plan):
    import ml_dtypes

    rows_per_core, nj, doc_order, widths = plan
    qm = np.ascontiguousarray(np.asarray(query_multi, np.float32))
    pm = np.ascontiguousarray(np.asarray(pos_multi, np.float32))
    qmask = np.asarray(q_mask).astype(bool)
    pmask = np.asarray(p_mask).astype(bool)

    pT = _prep_pos(pm, pmask, doc_order, widths)
    ident = np.eye(128, dtype=ml_dtypes.bfloat16)

    in_maps = []
    qohs = []
    for c in range(NCORES):
        qtok = np.zeros((nj * 128, D), np.float32)
        qoh = np.zeros((nj * 128, BPC), np.float32)
        pos = 0
        for i, b in enumerate(rows_per_core[c]):
            tok = qm[b][qmask[b]]
            n = len(tok)
            qtok[pos : pos + n] = tok
            qoh[pos : pos + n, i] = 1.0
            pos += n
        qT = np.ascontiguousarray(qtok.T).astype(ml_dtypes.bfloat16)
        in_maps.append({"pT": pT, "qT": qT, "identity": ident})
        qohs.append(qoh)  # [nj*128, BPC] host-side sum weights
    return in_maps, qohs


def _host_losses(dense_sim, S_late):
    """Float64 replica of the reference softmax/CE/KL tail."""

    def softmax_and_logp(z):
        m = z.max(axis=1, keepdims=True)
        e = np.exp(z - m)
        den = e.sum(axis=1, keepdims=True)
        return e / den, (z - m) - np.log(den)

    zd = dense_sim / TAU
    zl = S_late / TAU
    dp, logp_d = softmax_and_logp(zd)
    lp, logp_l = softmax_and_logp(zl)
    idx = np.arange(B)
    single = -logp_d[idx, idx].mean()
    multi = -logp_l[idx, idx].mean()
    kl = (dp * np.log((dp + EPS) / (lp + EPS))).sum(axis=1).mean()
    return single, multi, kl


def run(inputs: dict, trace: bool = False):
    """Run the spmd kernel; returns (loss tuple, BassKernelResults)."""
    qmask = np.asarray(inputs["q_mask"]).astype(bool)
    pmask = np.asarray(inputs["p_mask"]).astype(bool)
    plan = _plan(qmask, pmask)
    rows_per_core, nj, doc_order, widths = plan

    nc = _get_nc(nj, widths)
    in_maps, qohs = _prep_in_maps(
        inputs["query_multi"], inputs["pos_multi"], qmask, pmask, plan
    )
    res = run_bass_kernel_spmd(nc, in_maps, core_ids=list(range(NCORES)), trace=trace)

    # Assemble S_raw in original (row, doc) order.  Device output is
    # mx[slot_in_chunk, region, chunk, doc_in_region]; the masked sum over q
    # slots is a tiny host einsum.
    S_raw = np.zeros((B, B), np.float64)
    for c in range(NCORES):
        mx = np.asarray(res.results[c]["out"], np.float64)  # [128, NREG, nj, DPR]
        mx2 = mx.transpose(2, 0, 1, 3).reshape(nj * 128, B)  # [slot, sorted doc]
        block = qohs[c].T @ mx2  # [BPC, B]
        for i, b in enumerate(rows_per_core[c]):
            S_raw[b, doc_order] = block[i]

    t_i = np.maximum(qmask.sum(axis=1), 1).astype(np.float64)
    S_late = S_raw / t_i[:, None]

    qs = np.asarray(inputs["query_single"], np.float64)
    ps = np.asarray(inputs["pos_single"], np.float64)
    dense_sim = qs @ ps.T

    single, multi, kl = _host_losses(dense_sim, S_late)
    total = single + multi + kl
    out = (np.float32(total), np.float32(single), np.float32(multi), np.float32(kl))
    return out, res


def kernel(query_single, pos_single, query_multi, pos_multi, q_mask, p_mask):
    out, _ = run(
        {
            "query_single": query_single,
            "pos_single": pos_single,
            "query_multi": query_multi,
            "pos_multi": pos_multi,
            "q_mask": q_mask,
            "p_mask": p_mask,
        }
    )
    return out
